# revision 1
# baseline (speedup 1.0000x reference)
"""Trainium2 Bass kernel for nn_AxwinLowMixear (CSWin two-branch + global attention).

Sharding (8 cores): core = 2*b + role. Each core handles batch b:
  - CSWin branch `role` (96 output channels, all tokens, window-local order)
  - Global attention: slot0 = head (0 if role==0 else 2) full rows,
    slot1 = head 1 half rows (role0: rows 0:1568 natural, role1: rows 1568:3136
    via a 1568-token rotation of its xa copy so the compiled program is SPMD-uniform).
All per-core variation is carried in the input data (permuted xa copies, packed
weights); the Bass program is identical on every core.

Softmax normalization uses an appended ones-column in V (produced directly by
the qkv matmul via a constant-ones row smuggled into the projected activations),
so no partition-dim reductions are needed. All matmuls keep (128,128) PE tile
shape via zero padding. All constant fills arrive as host-DMA'd patterns; the
only gpsimd work is partition_broadcast for the softmax normalizers.
"""

import numpy as np
import ml_dtypes

B, DIM, RES, N = 4, 384, 56, 3136
TD, CSC = 192, 96
CS_SCALE = 48 ** -0.5
DN_SCALE = 64 ** -0.5
ROT = 1568
NJP = 3200          # global j padded (25 blocks of 128)
WPAD = 512          # cswin window j padded (4 blocks of 128)
NW = 8              # windows per image
WTOK = 392          # real tokens per window
VTW = 16 + NW * 448  # vt_cs width: (56,8)-padded images + edge pads

BF = ml_dtypes.bfloat16

_compiled = None


# ---------------------------------------------------------------- host prep --

def _cswin_perm(role):
    t = np.arange(N)
    w, rem = t // WTOK, t % WTOK
    r_, c_ = rem // 7, rem % 7
    if role == 0:
        return 56 * r_ + 7 * w + c_
    return 56 * (7 * w + c_) + r_


def _pad(a, rows, cols):
    out = np.zeros((rows, cols), np.float32)
    out[:a.shape[0], :a.shape[1]] = a
    return out.astype(BF)


def _host_consts():
    """Core-independent constant fills (zeros / ones patterns)."""
    m = {}
    m["ones448"] = np.ones((96, 448), BF)
    # dn[1] rows 64:128 : row 64 = ones over real tokens, 0 over j-pad
    d = np.zeros((64, NJP), np.float32)
    d[0, :N] = 1.0
    m["dn2i"] = d.astype(BF)
    # up[1] rows 64:128 : row 64 = ones over real window tokens, 0 over pad
    u = np.zeros((64, NW * WPAD), np.float32)
    for w in range(NW):
        u[0, w * WPAD:w * WPAD + WTOK] = 1.0
    m["up2i"] = u.astype(BF)
    return m


def _host_inputs(inputs, core, consts):
    b, role = core // 2, core % 2
    xa = np.asarray(inputs["xa"], np.float32).reshape(B, DIM, N)[b]
    qkv_up = np.asarray(inputs["qkv_up_w"], np.float32)
    qkv_dn = np.asarray(inputs["qkv_dn_w"], np.float32)
    perm_cs = _cswin_perm(role)
    rot = 0 if role == 0 else ROT
    perm_rot = (np.arange(N) + rot) % N

    m = dict(consts)
    m["xa_cs"] = xa[:, perm_cs].astype(BF)
    m["xa_gl"] = xa[:, perm_rot].astype(BF)
    m["wp1"] = _pad(np.asarray(inputs["proj1_w"], np.float32).T, 384, 256)
    m["wp2"] = _pad(np.asarray(inputs["proj2_w"], np.float32).T, 384, 256)

    base = role * 96
    wq = np.zeros((256, 128), np.float32)
    wq[:192, 0:48] = qkv_up[base:base + 48].T
    wq[:192, 64:112] = qkv_up[base + 48:base + 96].T
    m["wq_cs"] = wq.astype(BF)
    wk0 = np.zeros((256, 128), np.float32)
    wk0[:192, 0:48] = qkv_up[192 + base:192 + base + 48].T
    m["wk_cs0"] = wk0.astype(BF)
    wk1 = np.zeros((256, 128), np.float32)
    wk1[:192, 64:112] = qkv_up[192 + base + 48:192 + base + 96].T
    m["wk_cs1"] = wk1.astype(BF)
    # cswin v weights, layout [v_h0(48) 0(16) 1 | v_h1(48) 0(16) 1] = 130 cols;
    # the "1" columns pick up the constant-ones row (row 64 of up[1]).
    wv = np.zeros((256, 130), np.float32)
    wv[:192, 0:48] = qkv_up[384 + base:384 + base + 48].T
    wv[192, 64] = 1.0
    wv[:192, 65:113] = qkv_up[384 + base + 48:384 + base + 96].T
    wv[192, 129] = 1.0
    m["wv_cs"] = wv.astype(BF)
    m["wv_csT"] = _pad(qkv_up[384 + base:384 + base + 96].T, 256, 128)

    heads = (0, 1) if role == 0 else (2, 1)
    for s, h in enumerate(heads):
        m[f"wq_g{s}"] = _pad(qkv_dn[h * 64:(h + 1) * 64].T, 256, 128)
        m[f"wk_g{s}"] = _pad(qkv_dn[192 + h * 64:192 + (h + 1) * 64].T, 256, 128)
    wvg = np.zeros((256, 130), np.float32)
    wvg[:192, 0:64] = qkv_dn[384 + heads[0] * 64:384 + (heads[0] + 1) * 64].T
    wvg[192, 64] = 1.0
    wvg[:192, 65:129] = qkv_dn[384 + heads[1] * 64:384 + (heads[1] + 1) * 64].T
    wvg[192, 129] = 1.0
    m["wv_g"] = wvg.astype(BF)

    lw = np.asarray(inputs["lepe_w0" if role == 0 else "lepe_w1"], np.float32)[:, 0]
    lb = np.asarray(inputs["lepe_b0" if role == 0 else "lepe_b1"], np.float32)
    if role == 1:
        lw = lw.transpose(0, 2, 1)
    dl = np.zeros((10, 96, 128), np.float32)
    for tap in range(10):
        w_ = lw[:, tap // 3, tap % 3] if tap < 9 else lb
        dl[tap, 0:48, 0:48] = np.diag(w_[0:48])
        dl[tap, 48:96, 64:112] = np.diag(w_[48:96])
    m["dlepe"] = dl.astype(BF)
    return m


def _assemble(results, inputs):
    out = np.zeros((B, DIM, N), np.float32)
    for core in range(8):
        b, role = core // 2, core % 2
        part = np.asarray(results[core]["out_part"], np.float32)
        perm_cs = _cswin_perm(role)
        rot = 0 if role == 0 else ROT
        base = role * 96
        out[b, base:base + 96, perm_cs] = part[0:96].T
        h0 = 0 if role == 0 else 2
        out[b, 192 + h0 * 64:192 + (h0 + 1) * 64] = np.roll(part[96:160], rot, axis=1)
        if role == 0:
            out[b, 256:320, 0:ROT] = part[160:224, 0:ROT]
        else:
            out[b, 256:320, ROT:N] = part[160:224, 0:ROT]
    return out.reshape(B, DIM, RES, RES).astype(np.float32)


# ---------------------------------------------------------------- bass build --

def _build():
    import concourse.bacc as bacc
    import concourse.mybir as mybir
    import concourse.tile as tile
    import concourse.bass as bass

    fp32 = mybir.dt.float32
    bf16 = mybir.dt.bfloat16
    EXP = mybir.ActivationFunctionType.Exp
    CPY = mybir.ActivationFunctionType.Copy

    nc = bacc.Bacc("TRN2", target_bir_lowering=False, debug=False, num_devices=8)

    D = {}
    def din(name, shape):
        D[name] = nc.dram_tensor(name, shape, bf16, kind="ExternalInput")
    din("xa_cs", [DIM, N]); din("xa_gl", [DIM, N])
    din("wp1", [384, 256]); din("wp2", [384, 256])
    din("wq_cs", [256, 128]); din("wk_cs0", [256, 128])
    din("wk_cs1", [256, 128]); din("wv_cs", [256, 130])
    din("wv_csT", [256, 128])
    din("wq_g0", [256, 128]); din("wq_g1", [256, 128])
    din("wk_g0", [256, 128]); din("wk_g1", [256, 128])
    din("wv_g", [256, 130])
    din("dlepe", [10, 96, 128])
    din("ones448", [96, 448])
    din("dn2i", [64, NJP]); din("up2i", [64, NW * WPAD])
    out_part = nc.dram_tensor("out_part", [224, N], fp32, kind="ExternalOutput")

    with tile.TileContext(nc) as tc:
        with (
            tc.tile_pool(name="w", bufs=1) as wp,
            tc.tile_pool(name="act", bufs=1) as ap,
            tc.tile_pool(name="outp", bufs=2) as op,
            tc.tile_pool(name="nrm", bufs=2) as np_,
        ):
            # ---- weight loads ----
            W = {}
            for nm, chunks, cols in [
                ("wp1", 3, 256), ("wp2", 3, 256),
                ("wq_cs", 2, 128), ("wk_cs0", 2, 128),
                ("wk_cs1", 2, 128), ("wv_cs", 2, 130), ("wv_csT", 2, 128),
                ("wq_g0", 2, 128), ("wq_g1", 2, 128),
                ("wk_g0", 2, 128), ("wk_g1", 2, 128), ("wv_g", 2, 130),
            ]:
                tl = []
                for c in range(chunks):
                    t = wp.tile([128, cols], bf16, tag=f"{nm}{c}", name=f"{nm}{c}")
                    nc.sync.dma_start(t[:], D[nm][c * 128:(c + 1) * 128, :])
                    tl.append(t)
                W[nm] = tl
            dlepe_sb = wp.tile([96, 10 * 128], bf16, tag="dlepe", name="dlepe")
            nc.sync.dma_start(
                dlepe_sb[:].rearrange("p (t c) -> p t c", t=10),
                D["dlepe"][:].rearrange("t p c -> p t c"))
            ones_t = wp.tile([96, 448], bf16, tag="ones", name="ones")
            nc.sync.dma_start(ones_t[:], D["ones448"][:])

            # ---- persistent activation tiles ----
            qt_cs = ap.tile([128, N], bf16, tag="qt_cs", name="qt_cs")
            kcs = [ap.tile([128, NW * WPAD], bf16, tag=f"kcs{h}", name=f"kcs{h}") for h in range(2)]
            vt_cs = ap.tile([128, VTW], bf16, tag="vt_cs", name="vt_cs")
            vcs = ap.tile([128, NW * 4 * 130], bf16, tag="vcs", name="vcs")
            Q = [ap.tile([128, N], bf16, tag=f"Q{s}", name=f"Q{s}") for s in range(2)]
            K = [ap.tile([128, NJP], bf16, tag=f"K{s}", name=f"K{s}") for s in range(2)]
            V = ap.tile([128, 25 * 130], bf16, tag="V", name="V")

            # pad-region fills (cheap gpsimd memsets; data regions are
            # fully overwritten by the prep copies)
            for h in range(2):
                nc.gpsimd.memset(
                    kcs[h][:].rearrange("p (w c) -> p w c", c=WPAD)[:, :, WTOK:WPAD], 0.0)
            nc.gpsimd.memset(vt_cs[:, 0:8], 0.0)
            nc.gpsimd.memset(vt_cs[:, VTW - 8:VTW], 0.0)
            nc.gpsimd.memset(
                vt_cs[:, 8:VTW - 8].rearrange("p (x c) -> p x c", c=8)[:, :, 7:8], 0.0)
            nc.gpsimd.memset(K[0][:, N:NJP], 0.0)
            nc.gpsimd.memset(K[1][:, N:NJP], 0.0)

            with (
                tc.tile_pool(name="xap", bufs=1) as xap,
                tc.tile_pool(name="pprep", bufs=2, space=bass.MemorySpace.PSUM) as pp,
            ):
                # ---- xa + projection workspace loads ----
                xcs, xgl = [], []
                for c in range(3):
                    t = xap.tile([128, N], bf16, tag=f"xcs{c}", name=f"xcs{c}")
                    for kx in range(4):
                        nc.sync.dma_start(
                            t[:, kx * 784:(kx + 1) * 784],
                            D["xa_cs"][c * 128:(c + 1) * 128, kx * 784:(kx + 1) * 784])
                    xcs.append(t)
                for c in range(3):
                    t = xap.tile([128, N], bf16, tag=f"xgl{c}", name=f"xgl{c}")
                    for kx in range(4):
                        nc.sync.dma_start(
                            t[:, kx * 784:(kx + 1) * 784],
                            D["xa_gl"][c * 128:(c + 1) * 128, kx * 784:(kx + 1) * 784])
                    xgl.append(t)
                up = [xap.tile([128, NW * WPAD], bf16, tag=f"up{i}", name=f"up{i}") for i in range(2)]
                dn = [xap.tile([128, NJP], bf16, tag=f"dn{i}", name=f"dn{i}") for i in range(2)]
                nc.gpsimd.memset(
                    up[0][:].rearrange("p (w c) -> p w c", c=WPAD)[:, :, WTOK:WPAD], 0.0)
                nc.gpsimd.memset(
                    up[1][0:64, :].rearrange("p (w c) -> p w c", c=WPAD)[:, :, WTOK:WPAD], 0.0)
                nc.sync.dma_start(up[1][64:128, :], D["up2i"][:])
                nc.gpsimd.memset(dn[0][:, N:NJP], 0.0)
                nc.gpsimd.memset(dn[1][0:64, N:NJP], 0.0)
                nc.sync.dma_start(dn[1][64:128, :], D["dn2i"][:])

                # ---- P2: cswin prep ----
                for o in range(2):
                    for w in range(NW):
                        ps = pp.tile([128, 448], fp32, tag="proj", name="proj")
                        sl = slice(w * WTOK, (w + 1) * WTOK)
                        dsl = slice(w * WPAD, w * WPAD + WTOK)
                        for c in range(3):
                            nc.tensor.matmul(
                                ps[:, 0:WTOK], W["wp1"][c][:, o * 128:(o + 1) * 128],
                                xcs[c][:, sl], start=(c == 0), stop=(c == 2))
                        if o == 0:
                            nc.vector.tensor_copy(up[0][:, dsl], ps[:, 0:WTOK])
                        else:
                            nc.vector.tensor_copy(up[1][0:64, dsl], ps[0:64, 0:WTOK])
                for w in range(NW):
                    wsl = slice(w * WTOK, (w + 1) * WTOK)
                    psl = slice(w * WPAD, w * WPAD + WTOK)
                    ps = pp.tile([128, 448], fp32, tag="qk", name="qk")
                    for c in range(2):
                        nc.tensor.matmul(ps[:, 0:WTOK], W["wq_cs"][c][:],
                                         up[c][:, psl], start=(c == 0), stop=(c == 1))
                    nc.vector.tensor_copy(qt_cs[:, wsl], ps[:, 0:WTOK])
                    for h in range(2):
                        ps = pp.tile([128, 448], fp32, tag="qk", name="qk")
                        for c in range(2):
                            nc.tensor.matmul(ps[:, 0:WTOK], W[f"wk_cs{h}"][c][:],
                                             up[c][:, psl], start=(c == 0), stop=(c == 1))
                        nc.vector.tensor_copy(kcs[h][:, psl], ps[:, 0:WTOK])
                    # vT for lepe (both head blocks) into (56,8)-padded image
                    ps = pp.tile([128, 448], fp32, tag="qk", name="qk")
                    for c in range(2):
                        nc.tensor.matmul(ps[:, 0:WTOK], W["wv_csT"][c][:],
                                         up[c][:, psl], start=(c == 0), stop=(c == 1))
                    vdst = vt_cs[:, 8 + w * 448:8 + (w + 1) * 448] \
                        .rearrange("p (r c) -> p r c", c=8)[:, :, 0:7]
                    nc.vector.tensor_copy(
                        vdst, ps[:, 0:WTOK].rearrange("p (r c) -> p r c", c=7))
                    # v token-major with ones columns, single copy per block
                    for jb in range(4):
                        ps2 = pp.tile([128, 130], fp32, tag="vg", name="vg")
                        jsl = slice(w * WPAD + jb * 128, w * WPAD + (jb + 1) * 128)
                        for c in range(2):
                            nc.tensor.matmul(ps2[:], up[c][:, jsl],
                                             W["wv_cs"][c][:],
                                             start=(c == 0), stop=(c == 1))
                        vbase = (w * 4 + jb) * 130
                        nc.vector.tensor_copy(vcs[:, vbase:vbase + 130], ps2[:])

                # ---- P1: global prep ----
                # xa_dnT = wp2.T @ xa_gl ; dn[1] keeps its host ones-row (64:128)
                for o in range(2):
                    for nch in range(7):
                        ps = pp.tile([128, 448], fp32, tag="proj", name="proj")
                        sl = slice(nch * 448, (nch + 1) * 448)
                        for c in range(3):
                            nc.tensor.matmul(
                                ps[:], W["wp2"][c][:, o * 128:(o + 1) * 128],
                                xgl[c][:, sl], start=(c == 0), stop=(c == 2))
                        if o == 0:
                            nc.scalar.activation(dn[0][:, sl], ps[:], CPY)
                        else:
                            nc.scalar.activation(dn[1][0:64, sl], ps[0:64, :], CPY)
                # qT/kT per slot (copies on ACT: idle during prep)
                for s in range(2):
                    for nm, dst in ((f"wq_g{s}", Q[s]), (f"wk_g{s}", K[s])):
                        for nch in range(7):
                            ps = pp.tile([128, 448], fp32, tag="qk", name="qk")
                            sl = slice(nch * 448, (nch + 1) * 448)
                            for c in range(2):
                                nc.tensor.matmul(
                                    ps[:], W[nm][c][:], dn[c][:, sl],
                                    start=(c == 0), stop=(c == 1))
                            nc.scalar.activation(dst[:, sl], ps[:], CPY)
                # v for both slots + ones columns, single copy per block
                for jb in range(25):
                    ps = pp.tile([128, 130], fp32, tag="vg", name="vg")
                    sl = slice(jb * 128, (jb + 1) * 128)
                    for c in range(2):
                        nc.tensor.matmul(ps[:], dn[c][:, sl], W["wv_g"][c][:],
                                         start=(c == 0), stop=(c == 1))
                    nc.vector.tensor_copy(V[:, jb * 130:(jb + 1) * 130], ps[:])

            # ---- P4: cswin attention + lepe ----
            with (
                tc.tile_pool(name="ptcs", bufs=2) as ptcsp,
                tc.tile_pool(name="pscs", bufs=2, space=bass.MemorySpace.PSUM) as pscs,
                tc.tile_pool(name="pocs", bufs=2, space=bass.MemorySpace.PSUM) as pocs,
                tc.tile_pool(name="plep", bufs=2, space=bass.MemorySpace.PSUM) as plep,
            ):
                for w in range(NW):
                    wsl = slice(w * WTOK, (w + 1) * WTOK)
                    lp = plep.tile([128, 448], fp32, tag="lepe", name="lepe")
                    wbase = 8 + w * 448
                    nc.tensor.matmul(
                        lp[:, :], dlepe_sb[:, 4 * 128:5 * 128],
                        vt_cs[0:96, wbase:wbase + 448],
                        start=True, stop=False, skip_group_check=True)
                    for tap in range(9):
                        if tap == 4:
                            continue
                        dr, dc = tap // 3 - 1, tap % 3 - 1
                        r0, r1 = max(0, -dr), 56 - max(0, dr)
                        off, ln = r0 * 8, (r1 - r0) * 8
                        soff = wbase + (r0 + dr) * 8 + dc
                        nc.tensor.matmul(
                            lp[:, off:off + ln],
                            dlepe_sb[:, tap * 128:(tap + 1) * 128],
                            vt_cs[0:96, soff:soff + ln],
                            start=False, stop=False, skip_group_check=True)
                    nc.tensor.matmul(lp[:, :], dlepe_sb[:, 9 * 128:10 * 128],
                                     ones_t[:], start=False, stop=True,
                                     skip_group_check=True)
                    for h in range(2):
                        pts = []
                        for g in range(2):
                            ps = pscs.tile([128, 1024], fp32, tag="scs", name="scs")
                            for jj in range(2):
                                jb = g * 2 + jj
                                nc.tensor.matmul(
                                    ps[:, jj * 512:jj * 512 + WTOK],
                                    kcs[h][:, w * WPAD + jb * 128:w * WPAD + (jb + 1) * 128],
                                    qt_cs[:, wsl])
                            pt = ptcsp.tile([128, 2 * WTOK], bf16, tag=f"ptcs{g}", name=f"ptcs{g}")
                            ps3 = ps[:].rearrange("p (g c) -> p g c", c=512)[:, :, 0:WTOK]
                            pt3 = pt[:].rearrange("p (g c) -> p g c", c=WTOK)
                            nc.scalar.activation(pt3, ps3, EXP, scale=CS_SCALE)
                            pts.append(pt)
                        po = pocs.tile([128, WTOK], fp32, tag="ocs", name="ocs")
                        for jb in range(4):
                            vbase = (w * 4 + jb) * 130 + h * 65
                            nc.tensor.matmul(
                                po[0:65, :], vcs[:, vbase:vbase + 65],
                                pts[jb // 2][:, (jb % 2) * WTOK:(jb % 2 + 1) * WTOK],
                                start=(jb == 0), stop=(jb == 3))
                        r = np_.tile([1, WTOK], fp32, tag="rcs", name="rcs")
                        nc.vector.reciprocal(r[:], po[64:65, :])
                        rb = np_.tile([48, WTOK], fp32, tag="rbcs", name="rbcs")
                        nc.gpsimd.partition_broadcast(rb[:], r[:])
                        on = op.tile([48, WTOK], fp32, tag="ocs_sb", name="ocs_sb")
                        nc.vector.tensor_mul(on[:], po[0:48, :], rb[:])
                        fin = op.tile([48, WTOK], fp32, tag="fin_cs", name="fin_cs")
                        lp7 = lp[h * 64:h * 64 + 48, :] \
                            .rearrange("p (r c) -> p r c", c=8)[:, :, 0:7]
                        nc.vector.tensor_add(
                            fin[:].rearrange("p (r c) -> p r c", c=7),
                            on[:].rearrange("p (r c) -> p r c", c=7), lp7)
                        nc.sync.dma_start(
                            out_part[h * 48:(h + 1) * 48, wsl], fin[:])

            # ---- P3: global attention (software-pipelined) ----
            # Per 128-token j-block: phase-B matmuls of the PREVIOUS i-chunk
            # are emitted before the exp that overwrites that PT tile, so PE
            # keeps ACT fed and PT stays single-buffered.
            with (
                tc.tile_pool(name="pt", bufs=1) as ptp,
                tc.tile_pool(name="psg", bufs=2, space=bass.MemorySpace.PSUM) as psg,
                tc.tile_pool(name="pog", bufs=2, space=bass.MemorySpace.PSUM) as pog,
            ):
                jobs = [(0, 0, 1024), (0, 1024, 2048), (0, 2048, 3072),
                        (0, 3072, N), (1, 0, 1024), (1, 1024, ROT)]
                prev = None
                for job in jobs + [None]:
                    if job is not None:
                        s, i0, i1 = job
                        Wd = i1 - i0
                        subs = [(u, min(512, Wd - u)) for u in range(0, Wd, 512)]
                        po_subs = [pog.tile([128, 512], fp32, tag=f"og{k}", name=f"og{k}")
                                   for k in range(len(subs))]
                        pts = []
                    for jb in range(25):
                        if prev is not None:
                            ps_, psubs_, ppts, ppo, _pi0 = prev
                            for k, (u, sw) in enumerate(psubs_):
                                nc.tensor.matmul(
                                    ppo[k][0:65, 0:sw],
                                    V[:, jb * 130 + ps_ * 65:jb * 130 + ps_ * 65 + 65],
                                    ppts[jb][:, u:u + sw],
                                    start=(jb == 0), stop=(jb == 24))
                        if job is not None:
                            ps = psg.tile([128, 1024], fp32, tag="sg", name="sg")
                            for (u, sw) in subs:
                                nc.tensor.matmul(
                                    ps[:, u:u + sw],
                                    K[s][:, jb * 128:(jb + 1) * 128],
                                    Q[s][:, i0 + u:i0 + u + sw])
                            pt = ptp.tile([128, 1024], bf16, tag=f"ptg{jb}", name=f"ptg{jb}")
                            nc.scalar.activation(pt[:, 0:Wd], ps[:, 0:Wd], EXP,
                                                 scale=DN_SCALE)
                            pts.append(pt)
                    if prev is not None:
                        ps_, psubs_, ppts, ppo, pi0 = prev
                        for k, (u, sw) in enumerate(psubs_):
                            r = np_.tile([1, 512], fp32, tag="rg", name="rg")
                            nc.vector.reciprocal(r[0:1, 0:sw], ppo[k][64:65, 0:sw])
                            rb = np_.tile([64, 512], fp32, tag="rbg", name="rbg")
                            nc.gpsimd.partition_broadcast(rb[0:64, 0:sw], r[0:1, 0:sw])
                            on = op.tile([64, 512], fp32, tag="og_sb", name="og_sb")
                            nc.vector.tensor_mul(on[0:64, 0:sw], ppo[k][0:64, 0:sw],
                                                 rb[0:64, 0:sw])
                            nc.sync.dma_start(
                                out_part[96 + ps_ * 64:96 + (ps_ + 1) * 64,
                                         pi0 + u:pi0 + u + sw],
                                on[0:64, 0:sw])
                    prev = (s, subs, pts, po_subs, i0) if job is not None else None

    nc.compile()
    return nc


def kernel(**inputs) -> np.ndarray:
    global _compiled
    from concourse.bass_utils import run_bass_kernel_spmd
    if _compiled is None:
        _compiled = _build()
    nc = _compiled
    consts = _host_consts()
    in_maps = [_host_inputs(inputs, core, consts) for core in range(8)]
    res = run_bass_kernel_spmd(nc, in_maps, list(range(8)))
    return _assemble(res.results, inputs)

